# revision 1
# baseline (speedup 1.0000x reference)
"""Trainium2 Bass kernel for nn_BiaffineModule (biaffine span scorer).

Math (reference):
    x  = concat(final_hidden, feature_vecs)        [B,S,H+F]
    s  = x @ start_W + start_b                     [B,S,T]
    e  = x @ end_W + end_b                         [B,S,T]
    pre[b,s,e,c]  = sum_u (s @ U[:,c,:])[b,s,u] * e[b,e,u]
    ffn[b,s,e,c]  = (s@Ws)[b,s,c] + (e@We)[b,e,c] + (wh@Ww)[s,e,c] + lin_b[c]
    out = pre + ffn                                [B,S,S,C]

Sharding: the start axis `s` is split 8 ways (32 rows per core); each core
computes the full [B, 32, S, C] slab of the pairwise grid for all batches.
Small params are replicated; width_hidden is sliced per core.

Per-core dataflow (contraction dims live on SBUF partitions; the host
pre-transposes inputs so the device never transposes):
    sT   [256,128]   = sW_aug^T @ xT-slab-cols            (f32r)
    eT   [256,1024]  = eW_aug^T @ xT                      (f32r)
    sUT  [u, c*128+row] = sum_t U[t,c,u] sT[t,row]        (f32r)
    fsT  [16,128]    = Ws^T @ sT + lin_b x ones           (fp32)
    fw   [e,(c,s)]   = sum_w whT[w,e] Ww[w,c]             (bf16, K=64)
    out[e,(c,s)] per (b,e-chunk), PSUM-accumulated:
        2 MMs biaffine (eT x sUT), 2 MMs fe fold (eT x We-bcast),
        1 MM fs+lin_b fold (onesM x fs-row-in-zero-padded-K=128)
    then one DVE add of the fw plane and a contiguous store.

dtypes: float32r (same fp32 bits, single-pass TF32-like PE mode, ~1.5e-4
rel err) on all large matmuls; bf16 only for the small fw term (~1/16 of
output magnitude). End-to-end rel err vs fp32 reference ~2-4e-4.

Host unshards results[k][b,e,(c,s)] -> full[b, k*32+s, e, c].
"""

import sys

import numpy as np

sys.path.insert(0, "/opt/trn_rl_repo")

B, S, H, F = 4, 256, 768, 32
T, WD, C = 256, 64, 16
NCORES = 8
SLAB = S // NCORES          # 32 s-rows per core
ROWS = B * SLAB             # 128 slab rows (b-major, s-minor)
NB = B * S                  # 1024 xT columns (b-major, s-minor)
KPAD = 896                  # 7 * 128 (zero-padded H+F+1 ones-row)
NKCH = KPAD // 128

_CACHE = {}


def _build():
    import concourse.bacc as bacc
    import concourse.mybir as mybir
    from concourse import tile

    f32 = mybir.dt.float32
    f32r = mybir.dt.float32r
    bf16 = mybir.dt.bfloat16

    nc = bacc.Bacc(
        "TRN2", target_bir_lowering=False, debug=False, num_devices=NCORES
    )

    xT_d = nc.dram_tensor("xT", [KPAD, NB], f32r, kind="ExternalInput")
    sx_d = nc.dram_tensor("swxs", [KPAD, T + ROWS], f32r, kind="ExternalInput")
    eW_d = nc.dram_tensor("eW", [KPAD, T], f32r, kind="ExternalInput")
    U_d = nc.dram_tensor("U2", [T, C * T], f32r, kind="ExternalInput")
    wh_d = nc.dram_tensor("whT2", [WD, SLAB * S], bf16, kind="ExternalInput")
    lp_d = nc.dram_tensor("linPack", [128, 5 * C], f32r, kind="ExternalInput")
    lw_d = nc.dram_tensor("linWw", [WD, C], bf16, kind="ExternalInput")
    out_d = nc.dram_tensor("out", [B, S, SLAB * C], f32, kind="ExternalOutput")

    def chunked(dram, c0, c1):
        """[128, NKCH, c1-c0] AP over a column-slice of a [KPAD, w] tensor."""
        return dram.ap().rearrange("(n p) w -> p n w", p=128)[:, :, c0:c1]

    with tile.TileContext(nc) as tc:
        with (
            tc.tile_pool(name="consts", bufs=1) as consts,
            tc.tile_pool(name="acts", bufs=1) as acts,
            tc.tile_pool(name="outp", bufs=3) as outp,
            tc.tile_pool(name="pmm", bufs=2, space="PSUM") as pmm,
            tc.tile_pool(name="pmm2", bufs=2, space="PSUM") as pmm2,
            tc.tile_pool(name="ps5", bufs=3, space="PSUM") as ps5,
            tc.tile_pool(name="pfw", bufs=1, space="PSUM") as pfw,
        ):
            # ---- loads, in pipeline order --------------------------------------
            ones = consts.tile([1, 128], f32r)
            nc.vector.memset(ones[:].bitcast(f32), 1.0)
            onesM = consts.tile([128, 128], f32r)
            nc.vector.memset(onesM[:].bitcast(f32), 1.0)
            fsz = acts.tile([128, B, SLAB * C], f32r)
            nc.vector.memset(fsz[:].bitcast(f32), 0.0)

            # all loads on the Sync HWDGE ring, in consumption order; stores
            # and small SBUF gathers go on the Scalar ring (parallel issue).
            sxb = consts.tile([128, NKCH, T + ROWS], f32r)
            sxv = sx_d.ap().rearrange("(n p) w -> p n w", p=128)
            nc.sync.dma_start(sxb[:, 0:4, :], sxv[:, 0:4, :])
            nc.sync.dma_start(sxb[:, 4:NKCH, :], sxv[:, 4:NKCH, :])
            lpb = consts.tile([128, 5, C], f32r)
            nc.sync.dma_start(lpb[:], lp_d.ap().rearrange("p (n c) -> p n c", c=C))
            lwb = consts.tile([WD, C], bf16)
            nc.sync.dma_start(lwb[:], lw_d[:])
            whb = consts.tile([WD, SLAB * S], bf16)
            nc.sync.dma_start(whb[:], wh_d[:])
            xTb = consts.tile([128, NKCH, NB], f32r)
            Ub = consts.tile([128, 2, C * T], f32r)
            Uv = U_d.ap().rearrange("(n p) w -> p n w", p=128)
            nc.sync.dma_start(Ub[:, :, 0 : 4 * T], Uv[:, :, 0 : 4 * T])
            eWb = consts.tile([128, NKCH, T], f32r)
            nc.sync.dma_start(eWb[:], chunked(eW_d, 0, T))
            nc.sync.dma_start(Ub[:, :, 4 * T : 8 * T], Uv[:, :, 4 * T : 8 * T])
            nc.sync.dma_start(xTb[:, :, 0:512], chunked(xT_d, 0, 512))
            nc.sync.dma_start(Ub[:, :, 8 * T : 12 * T], Uv[:, :, 8 * T : 12 * T])
            nc.sync.dma_start(Ub[:, :, 12 * T : 16 * T], Uv[:, :, 12 * T : 16 * T])
            nc.sync.dma_start(xTb[:, :, 512:1024], chunked(xT_d, 512, 1024))

            # ---- sT [t, slab-row]  (f32r) --------------------------------------
            sT = acts.tile([128, 2, ROWS], f32r)
            for tch in range(2):
                ps = pmm.tile([128, ROWS], f32, tag="pmm")
                for k in range(NKCH):
                    nc.tensor.matmul(
                        ps[:],
                        sxb[:, k, tch * 128 : (tch + 1) * 128],
                        sxb[:, k, T : T + ROWS],
                        start=(k == 0),
                        stop=(k == NKCH - 1),
                    )
                nc.vector.tensor_copy(sT[:, tch, :], ps[:])

            # ---- fsT [c, slab-row] = Ws^T @ sT + lin_b x ones  (fp32) ----------
            fsT = acts.tile([16, ROWS], f32r)
            psf = pmm.tile([16, ROWS], f32, tag="pmm")
            for tch in range(2):
                nc.tensor.matmul(
                    psf[:], lpb[:, tch, :], sT[:, tch, :],
                    start=(tch == 0), stop=False,
                )
            nc.tensor.matmul(psf[:], lpb[0:1, 4, :], ones[:], start=False, stop=True)
            nc.vector.tensor_copy(fsT[:], psf[:])
            # fs row per b -> partition 0 of the zeroed fsz tile, (c,s) order
            for b in range(B):
                nc.scalar.dma_start(fsz[0:1, b, :], fsT[:, b * SLAB : (b + 1) * SLAB])

            # ---- fw plane [e, (c,s)] per e-chunk  (bf16, K=64) -----------------
            fw = acts.tile([128, 2, C, SLAB], f32)

            def fw_stage(ech):
                if True:
                    ps = pfw.tile([128, SLAB, C], f32, tag="pfw")
                    for s in range(SLAB):
                        nc.tensor.matmul(
                            ps[:, s, :],
                            whb[:, s * S + ech * 128 : s * S + ech * 128 + 128],
                            lwb[:],
                            start=True,
                            stop=True,
                        )
                    src = ps[:]
                    src = type(src)(
                        src.tensor, src.offset, [src.ap[0], [1, C], [C, SLAB]]
                    )
                    nc.vector.tensor_copy(fw[:, ech, :, :], src)

            # ---- fe planes: plane[b,ech][e,(c,s)] = fw[ech] + fe[b] bcast ------
            planes = acts.tile([128, B, 2, SLAB * C], f32)

            def fe_stage(b):
                for ech in range(2):
                    ecols = slice(b * S + ech * 128, b * S + ech * 128 + 128)
                    psq = pmm.tile([128, 16], f32, tag="pmm", name=f"feq{b}{ech}")
                    for tch in range(2):
                        nc.tensor.matmul(
                            psq[:],
                            eT[:, tch, ecols],
                            lpb[:, 2 + tch, :],
                            start=(tch == 0),
                            stop=(tch == 1),
                        )
                    feb = psq[:]
                    feb = type(feb)(feb.tensor, feb.offset, [feb.ap[0], [1, C], [0, SLAB]])
                    nc.vector.tensor_add(
                        planes[:, b, ech, :].rearrange("p (c s) -> p c s", c=C),
                        fw[:, ech, :, :],
                        feb,
                    )

            # ---- eT / sUT interleaved with load arrival (f32r) -----------------
            eT = acts.tile([128, 2, NB], f32r)
            sUT = [acts.tile([128, C, ROWS], f32r, name=f"sUT{u}") for u in range(2)]

            def eT_half(nch):
                for tch in range(2):
                    ps = pmm2.tile([128, 512], f32, tag="pmm2")
                    for k in range(NKCH):
                        nc.tensor.matmul(
                            ps[:],
                            eWb[:, k, tch * 128 : (tch + 1) * 128],
                            xTb[:, k, nch * 512 : (nch + 1) * 512],
                            start=(k == 0),
                            stop=(k == NKCH - 1),
                        )
                    nc.vector.tensor_copy(
                        eT[:, tch, nch * 512 : (nch + 1) * 512], ps[:]
                    )

            def sUT_group(grp):
                for uch in range(2):
                    ps = pmm.tile([128, 512], f32, tag="pmm")
                    for cl in range(4):
                        c = grp * 4 + cl
                        for tch in range(2):
                            nc.tensor.matmul(
                                ps[:, cl * 128 : (cl + 1) * 128],
                                Ub[:, tch, c * T + uch * 128 : c * T + uch * 128 + 128],
                                sT[:, tch, :],
                                start=(tch == 0),
                                stop=(tch == 1),
                            )
                    nc.vector.tensor_copy(
                        sUT[uch][:, grp * 4 : (grp + 1) * 4, :], ps[:]
                    )



            # ---- stage 5: biaffine + folds, assembly, store --------------------
            def stage5(b):
                for ech in range(2):
                    ps = ps5.tile([128, SLAB * C], f32, tag="ps5")
                    ecols = slice(b * S + ech * 128, b * S + ech * 128 + 128)
                    for uch in range(2):
                        rhs = sUT[uch][:, :, b * SLAB : (b + 1) * SLAB]
                        nc.tensor.matmul(
                            ps[:],
                            eT[:, uch, ecols],
                            rhs,
                            start=(uch == 0),
                            stop=False,
                        )
                    nc.tensor.matmul(
                        ps[:],
                        onesM[:],
                        fsz[:, b, :],
                        start=False,
                        stop=True,
                    )

                    ob = outp.tile([128, SLAB * C], f32, tag="outp")
                    nc.vector.tensor_add(ob[:], ps[:], planes[:, b, ech, :])
                    nc.scalar.dma_start(
                        out_d[b, ech * 128 : (ech + 1) * 128, :], ob[:]
                    )

            # emission order matched to DMA arrival order above
            fw_stage(0)
            sUT_group(0)
            fw_stage(1)
            sUT_group(1)
            eT_half(0)
            sUT_group(2)
            sUT_group(3)
            eT_half(1)
            fe_stage(0)
            fe_stage(1)
            fe_stage(2)
            fe_stage(3)
            stage5(0)
            stage5(1)
            stage5(2)
            stage5(3)

    nc.compile()
    return nc


def _get_nc():
    if "nc" not in _CACHE:
        _CACHE["nc"] = _build()
    return _CACHE["nc"]


def kernel(
    final_hidden, feature_vecs, start_W, start_b, end_W, end_b, U,
    width_hidden, lin_W, lin_b,
):
    import ml_dtypes

    from concourse.bass_utils import run_bass_kernel_spmd

    f32 = np.float32
    bf16 = ml_dtypes.bfloat16
    fh = np.asarray(final_hidden, f32)
    fv = np.asarray(feature_vecs, f32)

    x = np.concatenate([fh, fv], axis=-1)                  # [B,S,H+F]
    xT = np.zeros((KPAD, NB), f32)
    xT[: H + F] = x.reshape(NB, H + F).T
    xT[H + F] = 1.0                                        # bias fold row

    def aug(W, bvec):
        Wa = np.zeros((KPAD, T), f32)
        Wa[: H + F] = np.asarray(W, f32)
        Wa[H + F] = np.asarray(bvec, f32)
        return Wa

    sW = aug(start_W, start_b)
    eW = aug(end_W, end_b)
    U2 = np.ascontiguousarray(np.asarray(U, f32).reshape(T, C * T))
    linW = np.asarray(lin_W, f32)
    linWw = np.ascontiguousarray(linW[2 * T :]).astype(bf16)
    linPack = np.zeros((128, 5 * C), f32)
    linPack[:, 0:C] = linW[0:128, :]
    linPack[:, C : 2 * C] = linW[128:256, :]
    linPack[:, 2 * C : 3 * C] = linW[T : T + 128, :]
    linPack[:, 3 * C : 4 * C] = linW[T + 128 : 2 * T, :]
    linPack[0, 4 * C : 5 * C] = np.asarray(lin_b, f32)
    wh = np.asarray(width_hidden, f32)

    in_maps = []
    for k in range(NCORES):
        slab = wh[k * SLAB : (k + 1) * SLAB]               # [32, 256, 64]
        whT2 = np.ascontiguousarray(
            slab.transpose(2, 0, 1).reshape(WD, SLAB * S)
        ).astype(bf16)
        cols = (
            np.arange(B)[:, None] * S + (k * SLAB + np.arange(SLAB))[None, :]
        ).reshape(-1)
        swxs = np.ascontiguousarray(np.concatenate([sW, xT[:, cols]], axis=1))
        in_maps.append(
            {
                "xT": xT, "swxs": swxs, "eW": eW, "U2": U2,
                "whT2": whT2, "linPack": linPack, "linWw": linWw,
            }
        )

    _CACHE["last_in_maps"] = in_maps
    nc = _get_nc()
    res = run_bass_kernel_spmd(nc, in_maps, core_ids=list(range(NCORES)))

    full = np.empty((B, S, S, C), f32)
    for k in range(NCORES):
        r = res.results[k]["out"].reshape(B, S, C, SLAB)
        full[:, k * SLAB : (k + 1) * SLAB] = r.transpose(0, 3, 1, 2)
    return full



# revision 3
# speedup vs baseline: 1.3157x; 1.3157x over previous
"""Trainium2 Bass kernel for nn_BiaffineModule (biaffine span scorer).

Math (reference):
    x  = concat(final_hidden, feature_vecs)        [B,S,H+F]
    s  = x @ start_W + start_b                     [B,S,T]
    e  = x @ end_W + end_b                         [B,S,T]
    pre[b,s,e,c]  = sum_u (s @ U[:,c,:])[b,s,u] * e[b,e,u]
    ffn[b,s,e,c]  = (s@Ws)[b,s,c] + (e@We)[b,e,c] + (wh@Ww)[s,e,c] + lin_b[c]
    out = pre + ffn                                [B,S,S,C]

Sharding: the start axis `s` is split 8 ways (32 rows per core); each core
computes the full [B, 32, S, C] slab of the pairwise grid for all batches.
Small params + x are replicated; width_hidden and the x slab-columns are
sliced per core.

All large matmul operands are bf16 (1 cycle/row on the PE regardless of
free size, half the HBM bytes of f32); the width term is fp8e4m3 (it is
~1% of output magnitude, so fp8's ~4% rel err contributes ~4e-4).
Output is stored bf16 and upcast on the host. End-to-end rel err vs the
fp32 reference ~5e-3 (gate is 2e-2).

Per-core dataflow (contraction dims on SBUF partitions; host pre-
transposes so the device never transposes):
    sT   [t,row]     = sW_aug^T @ x-slab-cols              (bf16)
    eT   [t,(b,e)]   = eW_aug^T @ xT, computed per batch b (bf16)
    sUT  [u,(c,row)] = sum_t U[t,c,u] sT[t,row]            (bf16)
    fsT  [c,row]     = Ws^T @ sT + lin_b x ones            (K=1 fold)
    fw   [e,(c,s)]   = sum_w whT[w,e] Ww[w,c]              (fp8)
    fe   [e,c]       = We^T-fold of eT                     (bf16)
    out[e,(c,s)] per (b,e-chunk), PSUM-accumulated:
        2 MMs biaffine (eT x sUT) + 1 K=1 MM (ones x fs-row),
    then ob = ps + fw (DVE), ob2 = ob + fe-bcast (GpSimd), store bf16.

Schedule: U streams first (sUT -> stage5 is the serial tail), then xT
per batch; eT-b / fe-b / stage5-b are pipelined per batch so the output
tail overlaps the tail of the load stream. DMA issues are spread over
the sync/scalar rings; GpSimd does the fe broadcast adds.

Host unshards results[k][b,e,(c,s)] -> full[b, k*32+s, e, c].
"""

import sys

import numpy as np

sys.path.insert(0, "/opt/trn_rl_repo")

B, S, H, F = 4, 256, 768, 32
T, WD, C = 256, 64, 16
NCORES = 8
SLAB = S // NCORES          # 32 s-rows per core
ROWS = B * SLAB             # 128 slab rows (b-major, s-minor)
NB = B * S                  # 1024 xT columns (b-major, s-minor)
KPAD = 896                  # 7 * 128 (zero-padded H+F+1 ones-row)
NKCH = KPAD // 128

_CACHE = {}


def _build():
    import concourse.bacc as bacc
    import concourse.mybir as mybir
    from concourse import tile

    f32 = mybir.dt.float32
    bf16 = mybir.dt.bfloat16
    f8 = mybir.dt.float8e4

    nc = bacc.Bacc(
        "TRN2", target_bir_lowering=False, debug=False, num_devices=NCORES
    )

    sW_d = nc.dram_tensor("sWa", [KPAD, T], bf16, kind="ExternalInput")
    eW_d = nc.dram_tensor("eWa", [KPAD, T], bf16, kind="ExternalInput")
    xT_d = nc.dram_tensor("xTa", [KPAD, NB], bf16, kind="ExternalInput")
    xs_d = nc.dram_tensor("xsl", [KPAD, ROWS], bf16, kind="ExternalInput")
    U_d = nc.dram_tensor("U2", [T, C * T], bf16, kind="ExternalInput")
    wh_d = nc.dram_tensor("whT2", [WD, SLAB * S], f8, kind="ExternalInput")
    lw_d = nc.dram_tensor("linWw", [WD, C], f8, kind="ExternalInput")
    lp_d = nc.dram_tensor("linPack", [128, 5 * C], bf16, kind="ExternalInput")
    out_d = nc.dram_tensor("out", [B, S, SLAB * C], bf16, kind="ExternalOutput")

    def chunked(dram, c0, c1):
        """[128, NKCH, c1-c0] AP over a column-slice of a [KPAD, w] tensor."""
        return dram.ap().rearrange("(n p) w -> p n w", p=128)[:, :, c0:c1]

    with tile.TileContext(nc) as tc:
        with (
            tc.tile_pool(name="consts", bufs=1) as consts,
            tc.tile_pool(name="acts", bufs=1) as acts,
            tc.tile_pool(name="outp", bufs=3) as outp,
            tc.tile_pool(name="outp2", bufs=3) as outp2,
            tc.tile_pool(name="pmm", bufs=2, space="PSUM") as pmm,
            tc.tile_pool(name="pmm2", bufs=2, space="PSUM") as pmm2,
            tc.tile_pool(name="ps5", bufs=3, space="PSUM") as ps5,
            tc.tile_pool(name="pfw", bufs=1, space="PSUM") as pfw,
        ):
            ones1 = consts.tile([1, 128], bf16)
            nc.vector.memset(ones1[:], 1.0)

            # ---- loads ----------------------------------------------------
            # sync ring, in consumption order: sW+xsl (sT), U quarters
            # (sUT groups), eW, xT per-batch quarters (eT-b).
            sWb = consts.tile([128, NKCH, T], bf16)
            xSb = consts.tile([128, NKCH, ROWS], bf16)
            nc.sync.dma_start(sWb[:], chunked(sW_d, 0, T))
            nc.sync.dma_start(xSb[:], chunked(xs_d, 0, ROWS))

            Ub = consts.tile([128, 2, C * T], bf16)
            Uv = U_d.ap().rearrange("(n p) w -> p n w", p=128)
            for g in range(4):
                nc.sync.dma_start(
                    Ub[:, :, g * 4 * T : (g + 1) * 4 * T],
                    Uv[:, :, g * 4 * T : (g + 1) * 4 * T],
                )
            eWb = consts.tile([128, NKCH, T], bf16)
            nc.sync.dma_start(eWb[:], chunked(eW_d, 0, T))
            xTb = consts.tile([128, NKCH, NB], bf16)
            for b in range(B):
                nc.sync.dma_start(
                    xTb[:, :, b * 256 : (b + 1) * 256],
                    chunked(xT_d, b * 256, (b + 1) * 256),
                )

            # scalar ring: the small fw-path loads
            whb = consts.tile([WD, SLAB * S], f8)
            lwb = consts.tile([WD, C], f8)
            lpb = consts.tile([128, 5, C], bf16)
            nc.scalar.dma_start(whb[:], wh_d[:])
            nc.scalar.dma_start(lwb[:], lw_d[:])
            nc.scalar.dma_start(lpb[:], lp_d.ap().rearrange("p (n c) -> p n c", c=C))

            # ---- activations ---------------------------------------------
            sT = acts.tile([128, 2, ROWS], bf16)
            eT = acts.tile([128, 2, NB], bf16)
            sUT = [acts.tile([128, C, ROWS], bf16, name=f"sUT{u}") for u in range(2)]
            fw = acts.tile([128, 2, C, SLAB], bf16)
            fsT = acts.tile([16, ROWS], bf16)
            fsr = acts.tile([1, B, SLAB * C], bf16)
            feS = acts.tile([128, B, 2, C], bf16)

            # PE warmup: ramp the clock while loads stream (no input deps)
            wps = pmm.tile([128, 128], f32, tag="pmm", name="warm")
            for _ in range(8):
                nc.tensor.matmul(wps[:], ones1[:], ones1[:], start=True, stop=True)

            def sT_stage():
                for tch in range(2):
                    ps = pmm.tile([128, ROWS], f32, tag="pmm")
                    for k in range(NKCH):
                        nc.tensor.matmul(
                            ps[:],
                            sWb[:, k, tch * 128 : (tch + 1) * 128],
                            xSb[:, k, :],
                            start=(k == 0),
                            stop=(k == NKCH - 1),
                        )
                    nc.vector.tensor_copy(sT[:, tch, :], ps[:])

            def fw_stage(ech):
                ps = pfw.tile([128, SLAB, C], f32, tag="pfw")
                for s in range(SLAB):
                    nc.tensor.matmul(
                        ps[:, s, :],
                        whb[:, s * S + ech * 128 : s * S + ech * 128 + 128],
                        lwb[:],
                        start=True,
                        stop=True,
                    )
                src = ps[:]
                src = type(src)(
                    src.tensor, src.offset, [src.ap[0], [1, C], [C, SLAB]]
                )
                nc.vector.tensor_copy(fw[:, ech, :, :], src)

            def fsT_stage():
                psf = pmm.tile([16, ROWS], f32, tag="pmm")
                for tch in range(2):
                    nc.tensor.matmul(
                        psf[:], lpb[:, tch, :], sT[:, tch, :],
                        start=(tch == 0), stop=False,
                    )
                nc.tensor.matmul(
                    psf[:], lpb[0:1, 4, :], ones1[:], start=False, stop=True
                )
                nc.vector.tensor_copy(fsT[:], psf[:])
                for b in range(B):
                    nc.scalar.dma_start(
                        fsr[0:1, b, :], fsT[:, b * SLAB : (b + 1) * SLAB]
                    )

            def sUT_group(grp):
                for uch in range(2):
                    ps = pmm.tile([128, 512], f32, tag="pmm")
                    for cl in range(4):
                        c = grp * 4 + cl
                        for tch in range(2):
                            nc.tensor.matmul(
                                ps[:, cl * 128 : (cl + 1) * 128],
                                Ub[:, tch, c * T + uch * 128 : c * T + uch * 128 + 128],
                                sT[:, tch, :],
                                start=(tch == 0),
                                stop=(tch == 1),
                            )
                    dst = sUT[uch][:, grp * 4 : (grp + 1) * 4, :]
                    if grp % 2 == 0:
                        nc.vector.tensor_copy(dst, ps[:])
                    else:
                        nc.scalar.copy(dst, ps[:])

            def eT_b(b):
                for tch in range(2):
                    ps = pmm2.tile([128, 256], f32, tag="pmm2")
                    for k in range(NKCH):
                        nc.tensor.matmul(
                            ps[:],
                            eWb[:, k, tch * 128 : (tch + 1) * 128],
                            xTb[:, k, b * 256 : (b + 1) * 256],
                            start=(k == 0),
                            stop=(k == NKCH - 1),
                        )
                    nc.vector.tensor_copy(
                        eT[:, tch, b * 256 : (b + 1) * 256], ps[:]
                    )

            def fe_stage(b):
                for ech in range(2):
                    ecols = slice(b * S + ech * 128, b * S + ech * 128 + 128)
                    psq = pmm.tile([128, 16], f32, tag="pmm", name=f"feq{b}{ech}")
                    for tch in range(2):
                        nc.tensor.matmul(
                            psq[:],
                            eT[:, tch, ecols],
                            lpb[:, 2 + tch, :],
                            start=(tch == 0),
                            stop=(tch == 1),
                        )
                    nc.vector.tensor_copy(feS[:, b, ech, :], psq[:])

            def stage5(b):
                for ech in range(2):
                    ps = ps5.tile([128, SLAB * C], f32, tag="ps5")
                    ecols = slice(b * S + ech * 128, b * S + ech * 128 + 128)
                    for uch in range(2):
                        nc.tensor.matmul(
                            ps[:],
                            eT[:, uch, ecols],
                            sUT[uch][:, :, b * SLAB : (b + 1) * SLAB],
                            start=(uch == 0),
                            stop=False,
                        )
                    nc.tensor.matmul(
                        ps[:], ones1[:], fsr[0:1, b, :], start=False, stop=True
                    )

                    ob = outp.tile([128, SLAB * C], bf16, tag="outp")
                    nc.vector.tensor_add(
                        ob[:].rearrange("p (c s) -> p c s", c=C),
                        ps[:].rearrange("p (c s) -> p c s", c=C),
                        fw[:, ech, :, :],
                    )
                    ob2 = outp2.tile([128, SLAB * C], bf16, tag="outp2")
                    feb = feS[:, b, ech, :]
                    feb = type(feb)(
                        feb.tensor, feb.offset, [feb.ap[0], [1, C], [0, SLAB]]
                    )
                    nc.gpsimd.tensor_add(
                        ob2[:].rearrange("p (c s) -> p c s", c=C),
                        ob[:].rearrange("p (c s) -> p c s", c=C),
                        feb,
                    )
                    eng = nc.sync if (b * 2 + ech) % 2 == 0 else nc.scalar
                    eng.dma_start(
                        out_d[b, ech * 128 : (ech + 1) * 128, :], ob2[:]
                    )

            # emission order matched to DMA arrival order above
            sT_stage()
            fw_stage(0)
            fw_stage(1)
            fsT_stage()
            sUT_group(0)
            sUT_group(1)
            sUT_group(2)
            sUT_group(3)
            for b in range(B):
                eT_b(b)
                fe_stage(b)
                stage5(b)

    nc.compile()
    return nc


def _get_nc():
    if "nc" not in _CACHE:
        _CACHE["nc"] = _build()
    return _CACHE["nc"]


def kernel(
    final_hidden, feature_vecs, start_W, start_b, end_W, end_b, U,
    width_hidden, lin_W, lin_b,
):
    import ml_dtypes

    from concourse.bass_utils import run_bass_kernel_spmd

    f32 = np.float32
    bf16 = ml_dtypes.bfloat16
    f8 = ml_dtypes.float8_e4m3
    fh = np.asarray(final_hidden, f32)
    fv = np.asarray(feature_vecs, f32)

    x = np.concatenate([fh, fv], axis=-1)                  # [B,S,H+F]
    xT = np.zeros((KPAD, NB), f32)
    xT[: H + F] = x.reshape(NB, H + F).T
    xT[H + F] = 1.0                                        # bias fold row
    xTa = xT.astype(bf16)

    def aug(W, bvec):
        Wa = np.zeros((KPAD, T), f32)
        Wa[: H + F] = np.asarray(W, f32)
        Wa[H + F] = np.asarray(bvec, f32)
        return Wa.astype(bf16)

    sWa = aug(start_W, start_b)
    eWa = aug(end_W, end_b)
    U2 = np.ascontiguousarray(np.asarray(U, f32).reshape(T, C * T)).astype(bf16)
    linW = np.asarray(lin_W, f32)
    linWw = np.ascontiguousarray(linW[2 * T :]).astype(f8)
    linPack = np.zeros((128, 5 * C), f32)
    linPack[:, 0:C] = linW[0:128, :]
    linPack[:, C : 2 * C] = linW[128:256, :]
    linPack[:, 2 * C : 3 * C] = linW[T : T + 128, :]
    linPack[:, 3 * C : 4 * C] = linW[T + 128 : 2 * T, :]
    linPack[0, 4 * C : 5 * C] = np.asarray(lin_b, f32)
    linPack = linPack.astype(bf16)
    wh = np.asarray(width_hidden, f32)

    in_maps = []
    for k in range(NCORES):
        slab = wh[k * SLAB : (k + 1) * SLAB]               # [32, 256, 64]
        whT2 = np.ascontiguousarray(
            slab.transpose(2, 0, 1).reshape(WD, SLAB * S)
        ).astype(f8)
        cols = (
            np.arange(B)[:, None] * S + (k * SLAB + np.arange(SLAB))[None, :]
        ).reshape(-1)
        xsl = np.ascontiguousarray(xTa[:, cols])
        in_maps.append(
            {
                "xTa": xTa, "sWa": sWa, "eWa": eWa, "xsl": xsl, "U2": U2,
                "whT2": whT2, "linPack": linPack, "linWw": linWw,
            }
        )

    _CACHE["last_in_maps"] = in_maps
    nc = _get_nc()
    res = run_bass_kernel_spmd(nc, in_maps, core_ids=list(range(NCORES)))

    full = np.empty((B, S, S, C), f32)
    for k in range(NCORES):
        r = res.results[k]["out"].astype(f32).reshape(B, S, C, SLAB)
        full[:, k * SLAB : (k + 1) * SLAB] = r.transpose(0, 3, 1, 2)
    return full


# revision 4
# speedup vs baseline: 1.3444x; 1.0218x over previous
"""Trainium2 Bass kernel for nn_BiaffineModule (biaffine span scorer).

Math (reference):
    x  = concat(final_hidden, feature_vecs)        [B,S,H+F]
    s  = x @ start_W + start_b                     [B,S,T]
    e  = x @ end_W + end_b                         [B,S,T]
    pre[b,s,e,c]  = sum_u (s @ U[:,c,:])[b,s,u] * e[b,e,u]
    ffn[b,s,e,c]  = (s@Ws)[b,s,c] + (e@We)[b,e,c] + (wh@Ww)[s,e,c] + lin_b[c]
    out = pre + ffn                                [B,S,S,C]

Sharding: the start axis `s` is split 8 ways (32 rows per core); each core
computes the full [B, 32, S, C] slab of the pairwise grid for all batches.
Small params + x are replicated; width_hidden and the x slab-columns are
sliced per core.

All large matmul operands are bf16 (1 cycle/row on the PE regardless of
free size, half the HBM bytes of f32); the width term is fp8e4m3 (it is
~1% of output magnitude, so fp8's ~4% rel err contributes ~4e-4).
Output is stored bf16 and upcast on the host. End-to-end rel err vs the
fp32 reference ~5e-3 (gate is 2e-2).

Every DRAM input is host-packed into the exact SBUF tile layout
([128 partitions, per-partition bytes]) so each load is one DMA with
maximal contiguous runs (bf16 halves run lengths; without packing the
stream runs at half bandwidth).

Per-core dataflow (contraction dims on SBUF partitions):
    sT   [t,row]     = sW_aug^T @ x-slab-cols              (bf16)
    eT   [t,(b,e)]   = eW_aug^T @ xT, per b-pair halves    (bf16)
    sUT  [u,(c,row)] = sum_t U[t,c,u] sT[t,row]            (bf16)
    fsT  [c,row]     = Ws^T @ sT + lin_b x ones            (K=1 fold)
    fw   [e,(c,s)]   = sum_w whT[w,e] Ww[w,c]              (fp8)
    fe   [e,c]       = We^T-fold of eT                     (bf16)
    out[e,(c,s)] per (b,e-chunk), PSUM-accumulated:
        2 MMs biaffine (eT x sUT) + 1 K=1 MM (ones x fs-row),
    then ob = ps + fw (DVE), ob2 = ob + fe-bcast (GpSimd), store bf16.

Schedule: U streams first (sUT -> stage5 is the serial tail), then xT;
fw's 64 tiny matmuls run early under the load stream. DMA issues are
split across the sync/scalar rings; GpSimd does the fe broadcast adds.

Host unshards results[k][b,e,(c,s)] -> full[b, k*32+s, e, c].
"""

import sys

import numpy as np

sys.path.insert(0, "/opt/trn_rl_repo")

B, S, H, F = 4, 256, 768, 32
T, WD, C = 256, 64, 16
NCORES = 8
SLAB = S // NCORES          # 32 s-rows per core
ROWS = B * SLAB             # 128 slab rows (b-major, s-minor)
NB = B * S                  # 1024 xT columns (b-major, s-minor)
KPAD = 896                  # 7 * 128 (zero-padded H+F+1 ones-row)
NKCH = KPAD // 128

_CACHE = {}


def _build():
    import concourse.bacc as bacc
    import concourse.mybir as mybir
    from concourse import tile

    f32 = mybir.dt.float32
    bf16 = mybir.dt.bfloat16
    f8 = mybir.dt.float8e4

    nc = bacc.Bacc(
        "TRN2", target_bir_lowering=False, debug=False, num_devices=NCORES
    )

    # all inputs host-packed to SBUF layout: [128, per-partition words]
    sW_d = nc.dram_tensor("sWp", [128, NKCH * T], bf16, kind="ExternalInput")
    eW_d = nc.dram_tensor("eWp", [128, NKCH * T], bf16, kind="ExternalInput")
    xT_d = nc.dram_tensor("xTp", [128, B * NKCH * 256], bf16, kind="ExternalInput")
    xs_d = nc.dram_tensor("xsp", [128, NKCH * ROWS], bf16, kind="ExternalInput")
    U_d = nc.dram_tensor("Up", [128, 4 * 2 * 1024], bf16, kind="ExternalInput")
    wh_d = nc.dram_tensor("whT2", [WD, SLAB * S], f8, kind="ExternalInput")
    lw_d = nc.dram_tensor("linWw", [WD, C], f8, kind="ExternalInput")
    lp_d = nc.dram_tensor("linPack", [128, 5 * C], bf16, kind="ExternalInput")
    out_d = nc.dram_tensor("out", [B, S, SLAB * C], bf16, kind="ExternalOutput")

    with tile.TileContext(nc) as tc:
        with (
            tc.tile_pool(name="consts", bufs=1) as consts,
            tc.tile_pool(name="acts", bufs=1) as acts,
            tc.tile_pool(name="outp", bufs=3) as outp,
            tc.tile_pool(name="outp2", bufs=3) as outp2,
            tc.tile_pool(name="pmm", bufs=2, space="PSUM") as pmm,
            tc.tile_pool(name="pmm2", bufs=2, space="PSUM") as pmm2,
            tc.tile_pool(name="ps5", bufs=3, space="PSUM") as ps5,
            tc.tile_pool(name="pfw", bufs=1, space="PSUM") as pfw,
        ):
            ones1 = consts.tile([1, 128], bf16)
            nc.vector.memset(ones1[:], 1.0)

            # ---- loads (each a single contiguous-run DMA) -----------------
            # sync ring, consumption order: xsl+sW (sT), U quarters (sUT),
            # xT per-batch (eT).
            xSb = consts.tile([128, NKCH, ROWS], bf16)
            sWb = consts.tile([128, NKCH, T], bf16)
            nc.sync.dma_start(xSb[:], xs_d.ap().rearrange("p (n r) -> p n r", n=NKCH))
            nc.sync.dma_start(sWb[:], sW_d.ap().rearrange("p (n t) -> p n t", n=NKCH))

            Ub = consts.tile([128, 4, 2, 1024], bf16)
            Uv = U_d.ap().rearrange("p (g n w) -> p g n w", g=4, n=2)
            for g in range(4):
                nc.sync.dma_start(Ub[:, g], Uv[:, g])
            xTb = consts.tile([128, B, NKCH, 256], bf16)
            xTv = xT_d.ap().rearrange("p (b n w) -> p b n w", b=B, n=NKCH)
            for b in range(B):
                nc.sync.dma_start(xTb[:, b], xTv[:, b])

            # scalar ring: fw-path loads + eW
            whb = consts.tile([WD, SLAB * S], f8)
            lwb = consts.tile([WD, C], f8)
            lpb = consts.tile([128, 5, C], bf16)
            eWb = consts.tile([128, NKCH, T], bf16)
            nc.scalar.dma_start(whb[:], wh_d[:])
            nc.scalar.dma_start(lwb[:], lw_d[:])
            nc.scalar.dma_start(lpb[:], lp_d.ap().rearrange("p (n c) -> p n c", c=C))
            nc.scalar.dma_start(eWb[:], eW_d.ap().rearrange("p (n t) -> p n t", n=NKCH))

            # ---- activations ---------------------------------------------
            sT = acts.tile([128, 2, ROWS], bf16)
            eT = acts.tile([128, 2, NB], bf16)
            sUT = [acts.tile([128, C, ROWS], bf16, name=f"sUT{u}") for u in range(2)]
            fw = acts.tile([128, 2, C, SLAB], bf16)
            fsT = acts.tile([16, ROWS], bf16)
            fsr = acts.tile([1, B, SLAB * C], bf16)
            feS = acts.tile([128, B, 2, C], bf16)

            # PE warmup: ramp the clock while loads stream (no input deps)
            wps = pmm.tile([128, 128], f32, tag="pmm", name="warm")
            for _ in range(4):
                nc.tensor.matmul(wps[:], ones1[:], ones1[:], start=True, stop=True)

            def fw_stage(ech):
                ps = pfw.tile([128, SLAB, C], f32, tag="pfw")
                for s in range(SLAB):
                    nc.tensor.matmul(
                        ps[:, s, :],
                        whb[:, s * S + ech * 128 : s * S + ech * 128 + 128],
                        lwb[:],
                        start=True,
                        stop=True,
                    )
                src = ps[:]
                src = type(src)(
                    src.tensor, src.offset, [src.ap[0], [1, C], [C, SLAB]]
                )
                nc.vector.tensor_copy(fw[:, ech, :, :], src)

            def sT_stage():
                for tch in range(2):
                    ps = pmm.tile([128, ROWS], f32, tag="pmm")
                    for k in range(NKCH):
                        nc.tensor.matmul(
                            ps[:],
                            sWb[:, k, tch * 128 : (tch + 1) * 128],
                            xSb[:, k, :],
                            start=(k == 0),
                            stop=(k == NKCH - 1),
                        )
                    nc.vector.tensor_copy(sT[:, tch, :], ps[:])

            def fsT_stage():
                psf = pmm.tile([16, ROWS], f32, tag="pmm")
                for tch in range(2):
                    nc.tensor.matmul(
                        psf[:], lpb[:, tch, :], sT[:, tch, :],
                        start=(tch == 0), stop=False,
                    )
                nc.tensor.matmul(
                    psf[:], lpb[0:1, 4, :], ones1[:], start=False, stop=True
                )
                nc.vector.tensor_copy(fsT[:], psf[:])
                for b in range(B):
                    nc.scalar.dma_start(
                        fsr[0:1, b, :], fsT[:, b * SLAB : (b + 1) * SLAB]
                    )

            def sUT_group(grp):
                for uch in range(2):
                    ps = pmm.tile([128, 512], f32, tag="pmm")
                    for cl in range(4):
                        for tch in range(2):
                            nc.tensor.matmul(
                                ps[:, cl * 128 : (cl + 1) * 128],
                                Ub[:, grp, tch, cl * 256 + uch * 128 : cl * 256 + uch * 128 + 128],
                                sT[:, tch, :],
                                start=(tch == 0),
                                stop=(tch == 1),
                            )
                    dst = sUT[uch][:, grp * 4 : (grp + 1) * 4, :]
                    if grp % 2 == 0:
                        nc.vector.tensor_copy(dst, ps[:])
                    else:
                        nc.scalar.copy(dst, ps[:])

            def eT_half(h):
                # b-pair (2h, 2h+1); rhs free = [2 batches, 256 cols]
                for tch in range(2):
                    ps = pmm2.tile([128, 512], f32, tag="pmm2")
                    for k in range(NKCH):
                        nc.tensor.matmul(
                            ps[:],
                            eWb[:, k, tch * 128 : (tch + 1) * 128],
                            xTb[:, 2 * h : 2 * h + 2, k, :],
                            start=(k == 0),
                            stop=(k == NKCH - 1),
                        )
                    nc.vector.tensor_copy(
                        eT[:, tch, h * 512 : (h + 1) * 512], ps[:]
                    )

            def fe_stage(b):
                for ech in range(2):
                    ecols = slice(b * S + ech * 128, b * S + ech * 128 + 128)
                    psq = pmm.tile([128, 16], f32, tag="pmm", name=f"feq{b}{ech}")
                    for tch in range(2):
                        nc.tensor.matmul(
                            psq[:],
                            eT[:, tch, ecols],
                            lpb[:, 2 + tch, :],
                            start=(tch == 0),
                            stop=(tch == 1),
                        )
                    nc.vector.tensor_copy(feS[:, b, ech, :], psq[:])

            def stage5(b):
                for ech in range(2):
                    ps = ps5.tile([128, SLAB * C], f32, tag="ps5")
                    ecols = slice(b * S + ech * 128, b * S + ech * 128 + 128)
                    for uch in range(2):
                        nc.tensor.matmul(
                            ps[:],
                            eT[:, uch, ecols],
                            sUT[uch][:, :, b * SLAB : (b + 1) * SLAB],
                            start=(uch == 0),
                            stop=False,
                        )
                    nc.tensor.matmul(
                        ps[:], ones1[:], fsr[0:1, b, :], start=False, stop=True
                    )

                    ob = outp.tile([128, SLAB * C], bf16, tag="outp")
                    nc.vector.tensor_add(
                        ob[:].rearrange("p (c s) -> p c s", c=C),
                        ps[:].rearrange("p (c s) -> p c s", c=C),
                        fw[:, ech, :, :],
                    )
                    ob2 = outp2.tile([128, SLAB * C], bf16, tag="outp2")
                    feb = feS[:, b, ech, :]
                    feb = type(feb)(
                        feb.tensor, feb.offset, [feb.ap[0], [1, C], [0, SLAB]]
                    )
                    nc.gpsimd.tensor_add(
                        ob2[:].rearrange("p (c s) -> p c s", c=C),
                        ob[:].rearrange("p (c s) -> p c s", c=C),
                        feb,
                    )
                    eng = nc.sync if (b * 2 + ech) % 2 == 0 else nc.scalar
                    eng.dma_start(
                        out_d[b, ech * 128 : (ech + 1) * 128, :], ob2[:]
                    )

            # emission order matched to DMA arrival order above
            fw_stage(0)
            fw_stage(1)
            sT_stage()
            fsT_stage()
            sUT_group(0)
            sUT_group(1)
            sUT_group(2)
            sUT_group(3)
            eT_half(0)
            eT_half(1)
            for b in range(B):
                fe_stage(b)
                stage5(b)

    nc.compile()
    return nc


def _get_nc():
    if "nc" not in _CACHE:
        _CACHE["nc"] = _build()
    return _CACHE["nc"]


def _pack_kchunks(Wa):
    """[KPAD, w] -> [128, NKCH*w] in SBUF layout (partition p holds rows
    p, 128+p, ..., concatenated)."""
    w = Wa.shape[1]
    return np.ascontiguousarray(
        Wa.reshape(NKCH, 128, w).transpose(1, 0, 2).reshape(128, NKCH * w)
    )


def kernel(
    final_hidden, feature_vecs, start_W, start_b, end_W, end_b, U,
    width_hidden, lin_W, lin_b,
):
    import ml_dtypes

    from concourse.bass_utils import run_bass_kernel_spmd

    f32 = np.float32
    bf16 = ml_dtypes.bfloat16
    f8 = ml_dtypes.float8_e4m3
    fh = np.asarray(final_hidden, f32)
    fv = np.asarray(feature_vecs, f32)

    x = np.concatenate([fh, fv], axis=-1)                  # [B,S,H+F]
    xT = np.zeros((KPAD, NB), f32)
    xT[: H + F] = x.reshape(NB, H + F).T
    xT[H + F] = 1.0                                        # bias fold row
    xTa = xT.astype(bf16)
    # xTp[p, b, n, w] = xT[n*128+p, b*256+w]
    xTp = np.ascontiguousarray(
        xTa.reshape(NKCH, 128, B, 256).transpose(1, 2, 0, 3).reshape(128, -1)
    )

    def aug(W, bvec):
        Wa = np.zeros((KPAD, T), f32)
        Wa[: H + F] = np.asarray(W, f32)
        Wa[H + F] = np.asarray(bvec, f32)
        return _pack_kchunks(Wa.astype(bf16))

    sWp = aug(start_W, start_b)
    eWp = aug(end_W, end_b)
    U2 = np.asarray(U, f32).reshape(T, C * T).astype(bf16)
    # Up[p, g, n, w] = U2[n*128+p, g*1024+w]
    Up = np.ascontiguousarray(
        U2.reshape(2, 128, 4, 1024).transpose(1, 2, 0, 3).reshape(128, -1)
    )
    linW = np.asarray(lin_W, f32)
    linWw = np.ascontiguousarray(linW[2 * T :]).astype(f8)
    linPack = np.zeros((128, 5 * C), f32)
    linPack[:, 0:C] = linW[0:128, :]
    linPack[:, C : 2 * C] = linW[128:256, :]
    linPack[:, 2 * C : 3 * C] = linW[T : T + 128, :]
    linPack[:, 3 * C : 4 * C] = linW[T + 128 : 2 * T, :]
    linPack[0, 4 * C : 5 * C] = np.asarray(lin_b, f32)
    linPack = linPack.astype(bf16)
    wh = np.asarray(width_hidden, f32)

    in_maps = []
    for k in range(NCORES):
        slab = wh[k * SLAB : (k + 1) * SLAB]               # [32, 256, 64]
        whT2 = np.ascontiguousarray(
            slab.transpose(2, 0, 1).reshape(WD, SLAB * S)
        ).astype(f8)
        cols = (
            np.arange(B)[:, None] * S + (k * SLAB + np.arange(SLAB))[None, :]
        ).reshape(-1)
        xsp = np.ascontiguousarray(
            xTa[:, cols].reshape(NKCH, 128, ROWS).transpose(1, 0, 2).reshape(128, -1)
        )
        in_maps.append(
            {
                "xTp": xTp, "sWp": sWp, "eWp": eWp, "xsp": xsp, "Up": Up,
                "whT2": whT2, "linPack": linPack, "linWw": linWw,
            }
        )

    _CACHE["last_in_maps"] = in_maps
    nc = _get_nc()
    res = run_bass_kernel_spmd(nc, in_maps, core_ids=list(range(NCORES)))

    full = np.empty((B, S, S, C), f32)
    for k in range(NCORES):
        r = res.results[k]["out"].astype(f32).reshape(B, S, C, SLAB)
        full[:, k * SLAB : (k + 1) * SLAB] = r.transpose(0, 3, 1, 2)
    return full
